# revision 1
# baseline (speedup 1.0000x reference)
"""NT-Xent contrastive loss on 8 Trainium2 NeuronCores (Bass/Tile).

Strategy (no collectives -- measured ncfw latency floor ~85us makes the
all-gather hint design strictly worse):
  * Host pre-transposes embedded_data to embT [2048, 8192] (pure layout).
  * Slab cover: core c loads the 4 row-slabs S_c = {c, c+1, c+2, c+4} (mod 8)
    of emb (32 MiB/core). Every slab PAIR meets on some core (Z8 difference
    cover: slot-pairs at differences 1,2,3,4), so each of the 36 distinct
    1024x1024 blocks of the 8192x8192 similarity matrix is computed once
    globally; block (i,j) yields exp-row-sums for slab i (ACT accum) AND
    exp-col-sums for slab j (ones-matmul), exploiting sim symmetry.
  * Per core, uniform SPMD program: head matmul out_headT = W.T @ embT_slab
    (fp32r, 1 cyc/row), L2 normalize via ones-matmul normsq + Sqrt +
    reciprocal + K=1 broadcast matmul, then 5 sim blocks (diag + 4 pairs):
    psum [128,1024] fp32 -> ACT exp(10*x) with fused row-sum accum ->
    f32r exp tile -> ones-matmul col-sums. Diagonal exp values extracted
    exactly via a shifted-identity mask (mult+reduce) and subtracted on host.
  * pos term: elementwise product of slabs c and c+4 + ones-matmul -> the
    positive-pair similarities; log(pos) = 10*possim exactly (no exp needed).
  * Host (fp64): sums partial row/col contributions, subtracts diag,
    loss = -mean(10*possim - log(neg)).
"""
import numpy as np

SLOTS = [(c, (c + 1) % 8, (c + 2) % 8, (c + 4) % 8) for c in range(8)]
# blocks in local slot coords: (stationary, moving). B0 = diag.
BLOCKS = [(0, 0), (0, 1), (0, 2), (1, 3), (0, 3)]

_CACHE = {}


def _build():
    if "nc" in _CACHE:
        return _CACHE["nc"]
    import concourse.bacc as bacc
    import concourse.tile as tile
    import concourse.mybir as mybir

    F32, F32R = mybir.dt.float32, mybir.dt.float32r
    AF = mybir.ActivationFunctionType
    ALU = mybir.AluOpType

    nc = bacc.Bacc("TRN2", num_devices=8, debug=False)
    a_emb = nc.dram_tensor("embT", [2048, 4096], F32, kind="ExternalInput").ap()
    a_W = nc.dram_tensor("W", [2048, 256], F32, kind="ExternalInput").ap()
    a_b = nc.dram_tensor("b", [256], F32, kind="ExternalInput").ap()
    a_ones = nc.dram_tensor("ones", [128, 128], F32, kind="ExternalInput").ap()
    a_mask = nc.dram_tensor("mask", [128, 2048], F32, kind="ExternalInput").ap()
    o_rp = nc.dram_tensor("rowpart", [5, 1024], F32, kind="ExternalOutput").ap()
    o_cp = nc.dram_tensor("colpart", [4, 1024], F32, kind="ExternalOutput").ap()
    o_dg = nc.dram_tensor("diagexp", [1, 1024], F32, kind="ExternalOutput").ap()
    o_ps = nc.dram_tensor("possim", [1, 1024], F32, kind="ExternalOutput").ap()

    with tile.TileContext(nc) as tc:
        with tc.tile_pool(name="sb", bufs=1) as sb, \
             tc.tile_pool(name="emb", bufs=10) as embp, \
             tc.tile_pool(name="work", bufs=2) as wk, \
             tc.tile_pool(name="expp", bufs=3) as expp, \
             tc.tile_pool(name="headp", bufs=1, space="PSUM") as headp, \
             tc.tile_pool(name="simp", bufs=2, space="PSUM") as simp, \
             tc.tile_pool(name="csp", bufs=2, space="PSUM") as csp:

            t_W = sb.tile([128, 16, 256], F32R, name="t_W")
            nc.sync.dma_start(t_W[:], a_W.bitcast(F32R).rearrange("(kc p) d -> p kc d", p=128))
            t_b = sb.tile([128, 2], F32, name="t_b")
            nc.sync.dma_start(t_b[:], a_b.rearrange("(dh p) -> p dh", p=128))
            ones_col = sb.tile([128, 1], F32R, name="ones_col")
            nc.sync.dma_start(ones_col[:], a_ones.bitcast(F32R)[:, 0:1])
            ones_row = sb.tile([1, 128], F32, name="ones_row")
            nc.sync.dma_start(ones_row[:], a_ones[0:1, :])
            t_mask = sb.tile([128, 2048], F32, name="t_mask")
            nc.sync.dma_start(t_mask[:], a_mask[:])

            # staging accumulators
            rp_st = sb.tile([128, 5, 8], F32, name="rp_st")
            dg_st = sb.tile([128, 8], F32, name="dg_st")
            cp_st = sb.tile([1, 4096], F32, name="cp_st")
            ps_st = sb.tile([1, 1024], F32, name="ps_st")

            t_on = [sb.tile([128, 2, 1024], F32R, name=f"t_on{k}") for k in range(4)]

            def stage_a(k):
                t_h = wk.tile([128, 2, 1024], F32, name="t_h", tag="th")
                for h in range(2):
                    tes = []
                    for g in range(8):
                        t_e = embp.tile([128, 2, 512], F32R, name="t_e", tag="emb")
                        src = a_emb.bitcast(F32R)[256 * g:256 * (g + 1),
                                                  1024 * k + 512 * h:1024 * k + 512 * (h + 1)]
                        nc.sync.dma_start(t_e[:], src.rearrange("(c p) r -> p c r", p=128))
                        tes.append(t_e)
                    p_h = headp.tile([128, 2, 512], F32, name="p_h", tag="head")
                    for g in range(8):
                        for cc in range(2):
                            kk = 2 * g + cc
                            for dh in range(2):
                                nc.tensor.matmul(
                                    p_h[:, dh, :],
                                    t_W[:, kk, dh * 128:(dh + 1) * 128],
                                    tes[g][:, cc, :],
                                    start=(kk == 0), stop=(kk == 15),
                                )
                    for dh in range(2):
                        nc.vector.tensor_scalar_add(
                            t_h[:, dh, 512 * h:512 * (h + 1)], p_h[:, dh, :],
                            t_b[:, dh:dh + 1])
                t_sq = wk.tile([128, 2, 1024], F32R, name="t_sq", tag="sq")
                nc.vector.tensor_tensor(t_sq[:], t_h[:], t_h[:], ALU.mult)
                p_ns = [csp.tile([1, 512], F32, name=f"p_ns{nb}", tag="cs") for nb in range(2)]
                for nb in range(2):
                    for dh in range(2):
                        nc.tensor.matmul(p_ns[nb][:], ones_col[:],
                                         t_sq[:, dh, 512 * nb:512 * (nb + 1)],
                                         start=(dh == 0), stop=(dh == 1))
                t_nrm = wk.tile([1, 1024], F32, name="t_nrm", tag="nrm")
                for nb in range(2):
                    nc.scalar.activation(t_nrm[:, 512 * nb:512 * (nb + 1)], p_ns[nb][:], AF.Sqrt)
                t_ri = wk.tile([1, 1024], F32, name="t_ri", tag="ri")
                nc.vector.reciprocal(t_ri[:], t_nrm[:])
                p_bc = headp.tile([128, 2, 512], F32, name="p_bc", tag="head")
                for nb in range(2):
                    nc.tensor.matmul(p_bc[:, nb, :], ones_row[:],
                                     t_ri[:, 512 * nb:512 * (nb + 1)], start=True, stop=True)
                bc_flat = p_bc[:].rearrange("p a b -> p (a b)")
                for dh in range(2):
                    nc.vector.tensor_tensor(t_on[k][:, dh, :], t_h[:, dh, :], bc_flat, ALU.mult)

            def block(bslot, a, bm):
                p_cs = None
                if bslot > 0:
                    p_cs = [csp.tile([1, 512], F32, name=f"p_cs{bslot}_{nb}", tag="cs")
                            for nb in range(2)]
                for mb in range(8):
                    p_sim = simp.tile([128, 1024], F32, name="p_sim", tag="sim")
                    for dh in range(2):
                        for nb in range(2):
                            nc.tensor.matmul(
                                p_sim[:, 512 * nb:512 * (nb + 1)],
                                t_on[a][:, dh, 128 * mb:128 * (mb + 1)],
                                t_on[bm][:, dh, 512 * nb:512 * (nb + 1)],
                                start=(dh == 0), stop=(dh == 1))
                    t_exp = expp.tile([128, 1024], F32R, name="t_exp", tag="exp")
                    nc.scalar.activation(t_exp[:], p_sim[:], AF.Exp, scale=10.0,
                                         accum_out=rp_st[:, bslot, mb:mb + 1])
                    if bslot > 0:
                        for nb in range(2):
                            nc.tensor.matmul(p_cs[nb][:], ones_col[:],
                                             t_exp[:, 512 * nb:512 * (nb + 1)],
                                             start=(mb == 0), stop=(mb == 7))
                    else:
                        t_sc = expp.tile([128, 1024], F32, name="t_sc", tag="sc")
                        nc.vector.tensor_tensor(
                            t_sc[:], t_exp[:].bitcast(F32),
                            t_mask[:, 1024 - 128 * mb:2048 - 128 * mb], ALU.mult)
                        nc.vector.tensor_reduce(dg_st[:, mb:mb + 1], t_sc[:],
                                                mybir.AxisListType.X, ALU.add)
                if bslot > 0:
                    for nb in range(2):
                        nc.vector.tensor_copy(
                            cp_st[0:1, 1024 * (bslot - 1) + 512 * nb:
                                  1024 * (bslot - 1) + 512 * (nb + 1)], p_cs[nb][:])

            stage_a(0)
            block(0, 0, 0)
            stage_a(1)
            block(1, 0, 1)
            stage_a(2)
            block(2, 0, 2)
            stage_a(3)
            block(3, 1, 3)
            block(4, 0, 3)

            # pos: elementwise product slabs slot0 x slot3, column sums over d
            t_pp = wk.tile([128, 2, 1024], F32R, name="t_pp", tag="sq")
            for dh in range(2):
                nc.vector.tensor_tensor(t_pp[:, dh, :], t_on[0][:, dh, :].bitcast(F32),
                                        t_on[3][:, dh, :].bitcast(F32), ALU.mult)
            p_ps = [csp.tile([1, 512], F32, name=f"p_ps{nb}", tag="cs") for nb in range(2)]
            for nb in range(2):
                for dh in range(2):
                    nc.tensor.matmul(p_ps[nb][:], ones_col[:],
                                     t_pp[:, dh, 512 * nb:512 * (nb + 1)],
                                     start=(dh == 0), stop=(dh == 1))
                nc.vector.tensor_copy(ps_st[0:1, 512 * nb:512 * (nb + 1)], p_ps[nb][:])

            # final DMAs
            for bslot in range(5):
                nc.sync.dma_start(
                    o_rp[bslot:bslot + 1, :].rearrange("o (m p) -> p (o m)", p=128),
                    rp_st[:, bslot, :])
            nc.sync.dma_start(o_dg.rearrange("o (m p) -> p (o m)", p=128), dg_st[:])
            nc.sync.dma_start(o_cp.rearrange("a r -> (a r)")[None, :], cp_st[:])
            nc.sync.dma_start(o_ps[:], ps_st[:])

    nc.compile()
    _CACHE["nc"] = nc
    return nc


def _host_inputs(embedded_data, W, b):
    embT = np.ascontiguousarray(np.asarray(embedded_data, dtype=np.float32).T)
    W = np.asarray(W, dtype=np.float32)
    b = np.asarray(b, dtype=np.float32)
    mask = np.zeros((128, 2048), np.float32)
    mask[np.arange(128), np.arange(128) + 1024] = 1.0
    ones = np.ones((128, 128), np.float32)
    in_maps = []
    for c in range(8):
        cols = np.concatenate(
            [embT[:, 1024 * s:1024 * (s + 1)] for s in SLOTS[c]], axis=1)
        in_maps.append({"embT": np.ascontiguousarray(cols), "W": W, "b": b,
                        "ones": ones, "mask": mask})
    return in_maps


def _combine(results):
    neg = np.zeros(8192, np.float64)
    pos = np.zeros(8192, np.float64)
    for c in range(8):
        S = SLOTS[c]
        rp = results[c]["rowpart"].astype(np.float64)
        cp = results[c]["colpart"].astype(np.float64)
        dg = results[c]["diagexp"].astype(np.float64).ravel()
        sl = [np.s_[1024 * s:1024 * (s + 1)] for s in S]
        neg[sl[0]] += rp[0] - dg          # diag block, self-sim removed
        neg[sl[0]] += rp[1]; neg[sl[1]] += cp[0]   # B1 (0,1)
        neg[sl[0]] += rp[2]; neg[sl[2]] += cp[1]   # B2 (0,2)
        neg[sl[1]] += rp[3]; neg[sl[3]] += cp[2]   # B3 (1,3)
        if c < 4:                                   # B4 (0,3) dedup: cores 0-3
            neg[sl[0]] += rp[4]; neg[sl[3]] += cp[3]
            ps = results[c]["possim"].astype(np.float64).ravel()
            pos[sl[0]] = ps
            pos[1024 * S[3]:1024 * (S[3] + 1)] = ps
    loss = -np.mean(10.0 * pos - np.log(neg))
    return np.float32(loss)


def run(embedded_data, W, b, trace=False):
    from concourse import bass_utils
    nc = _build()
    in_maps = _host_inputs(embedded_data, W, b)
    res = bass_utils.run_bass_kernel_spmd(nc, in_maps, core_ids=list(range(8)),
                                          trace=trace)
    return _combine(res.results), res


def kernel(embedded_data, W, b):
    loss, _ = run(embedded_data, W, b, trace=False)
    return np.asarray(loss, dtype=np.float32)



# revision 2
# speedup vs baseline: 1.8143x; 1.8143x over previous
"""NT-Xent contrastive loss on 8 Trainium2 NeuronCores (Bass/Tile), v2.

Strategy (no collectives; slab-cover SPMD as v1, rebuilt for engine density):
  * Host pre-transposes embedded_data to embT [2048, 8192] and converts
    emb/W to bf16 (halves DMA; rel tol 2e-2 leaves ~100x margin).
  * Slab cover: core c loads slabs S_c = {c, c+1, c+2, c+4} (mod 8) of emb
    (16 MiB/core bf16).  Every slab pair meets on some core; each core
    computes 5 sim blocks of 1024x1024 (diag + 4 pairs).
  * Head: p_h = W.T @ embT chunkwise (bf16, FWL), +b into bf16 t_h.
  * Normalize: normsq via ones[128,128]-matmul (replicates norms across all
    128 partitions), then r = exp(-0.5*ln(normsq)) on ACT -- Ln and Exp share
    one table set, so the WHOLE kernel needs a single ACT table load.
    t_on = t_h * r (bf16).
  * Sim blocks: psum [128,1024] <- 4 bf16 matmuls; ACT exp(10x) with fused
    per-row accum (rowsums); colsums accumulated on DVE into [128,1024] f32
    tiles shipped to host (host reduces the 128 partitions).
  * Diag: sim_ii == 1 by construction, host subtracts e^10 (no mask pass).
  * pos: elementwise t_on0*t_on3 + ones-column matmul -> possim;
    log(pos) = 10*possim exactly.
  * Host (fp64) combines row/col partials; loss = -mean(10*possim - log(neg)).
"""
import numpy as np

SLOTS = [(c, (c + 1) % 8, (c + 2) % 8, (c + 4) % 8) for c in range(8)]
# blocks in local slot coords: (stationary, moving). B0 = diag.
BLOCKS = [(0, 0), (0, 1), (0, 2), (1, 3), (0, 3)]

_CACHE = {}


def _build():
    if "nc" in _CACHE:
        return _CACHE["nc"]
    import concourse.bacc as bacc
    import concourse.tile as tile
    import concourse.mybir as mybir

    F32, BF16 = mybir.dt.float32, mybir.dt.bfloat16
    AF = mybir.ActivationFunctionType
    ALU = mybir.AluOpType

    nc = bacc.Bacc("TRN2", num_devices=8, debug=False)
    a_emb = nc.dram_tensor("embT", [2048, 4096], BF16, kind="ExternalInput").ap()
    a_W = nc.dram_tensor("W", [2048, 256], BF16, kind="ExternalInput").ap()
    a_b = nc.dram_tensor("b", [256], F32, kind="ExternalInput").ap()
    a_ones = nc.dram_tensor("ones", [128, 128], BF16, kind="ExternalInput").ap()
    a_rs = nc.dram_tensor("rowacc", [128, 40], F32, kind="ExternalOutput").ap()
    a_cs = nc.dram_tensor("colacc", [128, 4096], F32, kind="ExternalOutput").ap()
    a_ps = nc.dram_tensor("possim", [1, 1024], F32, kind="ExternalOutput").ap()

    with tile.TileContext(nc) as tc:
        with tc.tile_pool(name="sb", bufs=1) as sb, \
             tc.tile_pool(name="emb", bufs=8) as embp, \
             tc.tile_pool(name="wk", bufs=2) as wk, \
             tc.tile_pool(name="cs", bufs=4) as csp, \
             tc.tile_pool(name="expp", bufs=3) as expp, \
             tc.tile_pool(name="headp", bufs=2, space="PSUM") as headp, \
             tc.tile_pool(name="nsp", bufs=1, space="PSUM") as nsp, \
             tc.tile_pool(name="simp", bufs=2, space="PSUM") as simp:

            t_ones = sb.tile([128, 128], BF16, name="t_ones")
            nc.sync.dma_start(t_ones[:], a_ones[:])
            t_b = sb.tile([128, 2], F32, name="t_b")
            nc.sync.dma_start(t_b[:], a_b.rearrange("(dh p) -> p dh", p=128))
            t_W = sb.tile([128, 16, 256], BF16, name="t_W")
            nc.sync.dma_start(t_W[:], a_W.rearrange("(kc p) d -> p kc d", p=128))

            rp_st = sb.tile([128, 5, 8], F32, name="rp_st")
            ps_st = sb.tile([1, 1024], F32, name="ps_st")
            t_on = [sb.tile([128, 2, 1024], BF16, name=f"t_on{k}") for k in range(4)]

            emb_tiles = {}

            def load_slab(k):
                tiles = []
                for g in range(4):
                    t_e = embp.tile([128, 4, 1024], BF16, name="t_e", tag="emb")
                    src = a_emb[512 * g:512 * (g + 1), 1024 * k:1024 * (k + 1)]
                    nc.sync.dma_start(t_e[:], src.rearrange("(kc p) r -> p kc r", p=128))
                    tiles.append(t_e)
                emb_tiles[k] = tiles

            def head_half(k, nb, t_h):
                for dh in range(2):
                    p_h = headp.tile([128, 512], F32, name="p_h", tag="head")
                    for kk in range(16):
                        g, kc = divmod(kk, 4)
                        nc.tensor.matmul(
                            p_h[:],
                            t_W[:, kk, 128 * dh:128 * (dh + 1)],
                            emb_tiles[k][g][:, kc, 512 * nb:512 * (nb + 1)],
                            start=(kk == 0), stop=(kk == 15),
                        )
                    nc.vector.tensor_scalar_add(
                        t_h[:, dh, 512 * nb:512 * (nb + 1)], p_h[:], t_b[:, dh:dh + 1])

            def sq_of(t_h):
                t_sq = wk.tile([128, 2, 1024], BF16, name="t_sq", tag="sq")
                nc.vector.tensor_tensor(t_sq[:], t_h[:], t_h[:], ALU.mult)
                return t_sq

            def ns_mm(t_sq):
                p_ns = nsp.tile([128, 1024], F32, name="p_ns", tag="ns")
                for nb in range(2):
                    for dh in range(2):
                        nc.tensor.matmul(
                            p_ns[:, 512 * nb:512 * (nb + 1)], t_ones[:],
                            t_sq[:, dh, 512 * nb:512 * (nb + 1)],
                            start=(dh == 0), stop=(dh == 1))
                return p_ns

            def norm_act(k, p_ns, t_h):
                t_ln = wk.tile([128, 1024], F32, name="t_ln", tag="ln")
                nc.scalar.activation(t_ln[:], p_ns[:], AF.Ln)
                t_r = wk.tile([128, 1024], F32, name="t_r", tag="r")
                nc.scalar.activation(t_r[:], t_ln[:], AF.Exp, scale=-0.5)
                for dh in range(2):
                    nc.vector.tensor_tensor(t_on[k][:, dh, :], t_h[:, dh, :],
                                            t_r[:], ALU.mult)

            def block(bslot, a, bm):
                t_cs = None
                if bslot > 0:
                    t_cs = csp.tile([128, 1024], F32, name=f"t_cs{bslot}", tag="cs")
                for mb in range(8):
                    p_sim = simp.tile([128, 1024], F32, name="p_sim", tag="sim")
                    for dh in range(2):
                        for nb in range(2):
                            nc.tensor.matmul(
                                p_sim[:, 512 * nb:512 * (nb + 1)],
                                t_on[a][:, dh, 128 * mb:128 * (mb + 1)],
                                t_on[bm][:, dh, 512 * nb:512 * (nb + 1)],
                                start=(dh == 0), stop=(dh == 1))
                    t_exp = expp.tile([128, 1024], BF16, name="t_exp", tag="exp")
                    nc.scalar.activation(t_exp[:], p_sim[:], AF.Exp, scale=10.0,
                                         accum_out=rp_st[:, bslot, mb:mb + 1])
                    if bslot > 0:
                        if mb == 0:
                            nc.vector.tensor_copy(t_cs[:], t_exp[:])
                        else:
                            nc.vector.tensor_tensor(t_cs[:], t_cs[:], t_exp[:], ALU.add)
                if bslot > 0:
                    nc.sync.dma_start(a_cs[:, 1024 * (bslot - 1):1024 * bslot], t_cs[:])

            t_h = [None] * 4
            t_sq = [None] * 4
            p_ns = [None] * 4

            def new_th(k):
                t_h[k] = wk.tile([128, 2, 1024], BF16, name=f"t_h{k}", tag="th")

            # ---- emission order tuned to keep the PE stream dense ----
            load_slab(0)
            load_slab(1)
            new_th(0)
            head_half(0, 0, t_h[0])
            head_half(0, 1, t_h[0])
            t_sq[0] = sq_of(t_h[0])
            load_slab(2)
            new_th(1)
            head_half(1, 0, t_h[1])
            p_ns[0] = ns_mm(t_sq[0])
            norm_act(0, p_ns[0], t_h[0])
            head_half(1, 1, t_h[1])
            t_sq[1] = sq_of(t_h[1])
            block(0, *BLOCKS[0])
            load_slab(3)
            new_th(2)
            head_half(2, 0, t_h[2])
            p_ns[1] = ns_mm(t_sq[1])
            norm_act(1, p_ns[1], t_h[1])
            head_half(2, 1, t_h[2])
            t_sq[2] = sq_of(t_h[2])
            block(1, *BLOCKS[1])
            new_th(3)
            head_half(3, 0, t_h[3])
            p_ns[2] = ns_mm(t_sq[2])
            norm_act(2, p_ns[2], t_h[2])
            head_half(3, 1, t_h[3])
            t_sq[3] = sq_of(t_h[3])
            p_ns[3] = ns_mm(t_sq[3])
            norm_act(3, p_ns[3], t_h[3])
            block(2, *BLOCKS[2])
            block(3, *BLOCKS[3])
            block(4, *BLOCKS[4])

            # pos: elementwise product slabs slot0 x slot3, column sums over d
            t_pp = wk.tile([128, 2, 1024], BF16, name="t_pp", tag="pp", bufs=1)
            nc.vector.tensor_tensor(t_pp[:], t_on[0][:], t_on[3][:], ALU.mult)
            for nb in range(2):
                p_ps = headp.tile([1, 512], F32, name=f"p_ps{nb}", tag="head")
                for dh in range(2):
                    nc.tensor.matmul(p_ps[:], t_ones[:, 0:1],
                                     t_pp[:, dh, 512 * nb:512 * (nb + 1)],
                                     start=(dh == 0), stop=(dh == 1))
                nc.vector.tensor_copy(ps_st[0:1, 512 * nb:512 * (nb + 1)], p_ps[:])

            # final DMAs
            nc.sync.dma_start(a_rs[:], rp_st[:].rearrange("p a m -> p (a m)"))
            nc.sync.dma_start(a_ps[:], ps_st[:])

    nc.compile()
    _CACHE["nc"] = nc
    return nc


def _host_inputs(embedded_data, W, b):
    import ml_dtypes
    bf16 = ml_dtypes.bfloat16
    embT = np.asarray(embedded_data, dtype=np.float32).T.astype(bf16)
    Wb = np.asarray(W, dtype=np.float32).astype(bf16)
    b32 = np.asarray(b, dtype=np.float32)
    ones = np.ones((128, 128), dtype=bf16)
    in_maps = []
    for c in range(8):
        cols = np.ascontiguousarray(np.concatenate(
            [embT[:, 1024 * s:1024 * (s + 1)] for s in SLOTS[c]], axis=1))
        in_maps.append({"embT": cols, "W": Wb, "b": b32, "ones": ones})
    return in_maps


def _combine(results):
    neg = np.zeros(8192, np.float64)
    pos = np.zeros(8192, np.float64)
    E10 = np.exp(10.0)
    for c in range(8):
        S = SLOTS[c]
        rs = results[c]["rowacc"].astype(np.float64).reshape(128, 5, 8)
        cs = results[c]["colacc"].astype(np.float64).reshape(128, 4, 1024).sum(axis=0)
        rows = [rs[:, bl, :].T.reshape(-1) for bl in range(5)]
        sl = [np.s_[1024 * s:1024 * (s + 1)] for s in S]
        neg[sl[0]] += rows[0] - E10          # diag block, self-sim removed
        neg[sl[0]] += rows[1]; neg[sl[1]] += cs[0]   # B1 (0,1)
        neg[sl[0]] += rows[2]; neg[sl[2]] += cs[1]   # B2 (0,2)
        neg[sl[1]] += rows[3]; neg[sl[3]] += cs[2]   # B3 (1,3)
        if c < 4:                                     # B4 (0,3) dedup: cores 0-3
            neg[sl[0]] += rows[4]; neg[sl[3]] += cs[3]
            ps = results[c]["possim"].astype(np.float64).ravel()
            pos[sl[0]] = ps
            pos[1024 * S[3]:1024 * (S[3] + 1)] = ps
    loss = -np.mean(10.0 * pos - np.log(neg))
    return np.float32(loss)


def run(embedded_data, W, b, trace=False):
    from concourse import bass_utils
    nc = _build()
    in_maps = _host_inputs(embedded_data, W, b)
    res = bass_utils.run_bass_kernel_spmd(nc, in_maps, core_ids=list(range(8)),
                                          trace=trace)
    return _combine(res.results), res


def kernel(embedded_data, W, b):
    loss, _ = run(embedded_data, W, b, trace=False)
    return np.asarray(loss, dtype=np.float32)


# revision 5
# speedup vs baseline: 1.8403x; 1.0143x over previous
"""NT-Xent contrastive loss on 8 Trainium2 NeuronCores (Bass/Tile), v2.

Strategy (no collectives; slab-cover SPMD as v1, rebuilt for engine density):
  * Host pre-transposes embedded_data to embT [2048, 8192] and converts
    emb/W to bf16 (halves DMA; rel tol 2e-2 leaves ~100x margin).
  * Slab cover: core c loads slabs S_c = {c, c+1, c+2, c+4} (mod 8) of emb
    (16 MiB/core bf16).  Every slab pair meets on some core; each core
    computes 5 sim blocks of 1024x1024 (diag + 4 pairs).
  * Head: p_h = W.T @ embT chunkwise (bf16, FWL), +b into bf16 t_h.
  * Normalize: normsq via ones[128,128]-matmul (replicates norms across all
    128 partitions), then r = exp(-0.5*ln(normsq)) on ACT -- Ln and Exp share
    one table set, so the WHOLE kernel needs a single ACT table load.
    t_on = t_h * r (bf16).
  * Sim blocks: psum [128,1024] <- 4 bf16 matmuls; ACT exp(10x) with fused
    per-row accum (rowsums); colsums accumulated on DVE into [128,1024] f32
    tiles shipped to host (host reduces the 128 partitions).
  * Diag: sim_ii == 1 by construction, host subtracts e^10 (no mask pass).
  * pos: elementwise t_on0*t_on3 + ones-column matmul -> possim;
    log(pos) = 10*possim exactly.
  * Host (fp64) combines row/col partials; loss = -mean(10*possim - log(neg)).
"""
import numpy as np

SLOTS = [(c, (c + 1) % 8, (c + 2) % 8, (c + 4) % 8) for c in range(8)]
# blocks in local slot coords: (stationary, moving). B0 = diag.
BLOCKS = [(0, 0), (0, 1), (0, 2), (1, 3), (0, 3)]

_CACHE = {}


def _build():
    if "nc" in _CACHE:
        return _CACHE["nc"]
    import concourse.bacc as bacc
    import concourse.tile as tile
    import concourse.mybir as mybir

    F32, BF16 = mybir.dt.float32, mybir.dt.bfloat16
    AF = mybir.ActivationFunctionType
    ALU = mybir.AluOpType

    nc = bacc.Bacc("TRN2", num_devices=8, debug=False)

    # Pin Ln+Exp to the one table set containing both: strip them from every
    # other set in the (cached) table dict so the table-load inserter cannot
    # alternate between exp_and_others and natural_log (saves ~2.7us per
    # switch and keeps the normalize chain off the ACT critical path).
    # Key order is preserved, so act_func_set_id indices stay valid.
    tables = bacc.get_activation_tables(nc.m.arch)
    if "natural_log_exp_and_others" in tables:
        for name, funcs in tables.items():
            if name != "natural_log_exp_and_others":
                funcs.discard(AF.Exp)
                funcs.discard(AF.Ln)

    a_emb = nc.dram_tensor("embT", [2048, 4096], BF16, kind="ExternalInput").ap()
    a_W = nc.dram_tensor("W", [2048, 256], BF16, kind="ExternalInput").ap()
    a_b = nc.dram_tensor("b", [256], F32, kind="ExternalInput").ap()
    a_ones = nc.dram_tensor("ones", [128, 128], BF16, kind="ExternalInput").ap()
    a_rs = nc.dram_tensor("rowacc", [128, 40], F32, kind="ExternalOutput").ap()
    a_cs = nc.dram_tensor("colacc", [128, 4096], F32, kind="ExternalOutput").ap()
    a_ps = nc.dram_tensor("possim", [1, 1024], F32, kind="ExternalOutput").ap()

    with tile.TileContext(nc) as tc:
        with tc.tile_pool(name="sb", bufs=1) as sb, \
             tc.tile_pool(name="emb", bufs=8) as embp, \
             tc.tile_pool(name="wk", bufs=2) as wk, \
             tc.tile_pool(name="cs", bufs=4) as csp, \
             tc.tile_pool(name="expp", bufs=3) as expp, \
             tc.tile_pool(name="headp", bufs=2, space="PSUM") as headp, \
             tc.tile_pool(name="nsp", bufs=1, space="PSUM") as nsp, \
             tc.tile_pool(name="simp", bufs=2, space="PSUM") as simp:

            t_ones = sb.tile([128, 128], BF16, name="t_ones")
            nc.sync.dma_start(t_ones[:], a_ones[:])
            t_b = sb.tile([128, 2], F32, name="t_b")
            nc.sync.dma_start(t_b[:], a_b.rearrange("(dh p) -> p dh", p=128))
            t_W = sb.tile([128, 16, 256], BF16, name="t_W")
            for wc in range(4):
                nc.sync.dma_start(
                    t_W[:, 4 * wc:4 * (wc + 1), :],
                    a_W[512 * wc:512 * (wc + 1), :].rearrange(
                        "(kc p) d -> p kc d", p=128))

            # HAM warm-up: ~4.5us of dummy matmuls on the ones tile while the
            # first slab DMAs land, so the PE clock gate opens (1.2->2.4 GHz)
            # before the real head matmuls begin.
            p_wu = headp.tile([128, 512], F32, name="p_wu", tag="head")
            for _ in range(42):
                nc.tensor.matmul(p_wu[:, 0:128], t_ones[:], t_ones[:],
                                 start=True, stop=True)

            rp_st = sb.tile([128, 5, 8], F32, name="rp_st")
            ps_st = sb.tile([1, 1024], F32, name="ps_st")
            t_on = [sb.tile([128, 2, 1024], BF16, name=f"t_on{k}") for k in range(4)]

            emb_tiles = {}

            def load_slab(k):
                tiles = []
                for g in range(4):
                    t_e = embp.tile([128, 4, 1024], BF16, name="t_e", tag="emb")
                    src = a_emb[512 * g:512 * (g + 1), 1024 * k:1024 * (k + 1)]
                    nc.sync.dma_start(t_e[:], src.rearrange("(kc p) r -> p kc r", p=128))
                    tiles.append(t_e)
                emb_tiles[k] = tiles

            def head_half(k, nb, t_h):
                for dh in range(2):
                    p_h = headp.tile([128, 512], F32, name="p_h", tag="head")
                    for kk in range(16):
                        g, kc = divmod(kk, 4)
                        nc.tensor.matmul(
                            p_h[:],
                            t_W[:, kk, 128 * dh:128 * (dh + 1)],
                            emb_tiles[k][g][:, kc, 512 * nb:512 * (nb + 1)],
                            start=(kk == 0), stop=(kk == 15),
                        )
                    nc.vector.tensor_scalar_add(
                        t_h[:, dh, 512 * nb:512 * (nb + 1)], p_h[:], t_b[:, dh:dh + 1])

            def sq_of(t_h):
                t_sq = wk.tile([128, 2, 1024], BF16, name="t_sq", tag="sq")
                nc.vector.tensor_tensor(t_sq[:], t_h[:], t_h[:], ALU.mult)
                return t_sq

            def ns_mm(t_sq):
                p_ns = nsp.tile([128, 1024], F32, name="p_ns", tag="ns")
                for nb in range(2):
                    for dh in range(2):
                        nc.tensor.matmul(
                            p_ns[:, 512 * nb:512 * (nb + 1)], t_ones[:],
                            t_sq[:, dh, 512 * nb:512 * (nb + 1)],
                            start=(dh == 0), stop=(dh == 1))
                return p_ns

            def norm_act(k, p_ns, t_h):
                t_ln = wk.tile([128, 1024], F32, name="t_ln", tag="ln")
                nc.scalar.activation(t_ln[:], p_ns[:], AF.Ln)
                t_r = wk.tile([128, 1024], F32, name="t_r", tag="r")
                nc.scalar.activation(t_r[:], t_ln[:], AF.Exp, scale=-0.5)
                for dh in range(2):
                    nc.vector.tensor_tensor(t_on[k][:, dh, :], t_h[:, dh, :],
                                            t_r[:], ALU.mult)

            def block(bslot, a, bm):
                t_cs = None
                if bslot > 0:
                    t_cs = csp.tile([128, 1024], F32, name=f"t_cs{bslot}", tag="cs")
                for mb in range(8):
                    p_sim = simp.tile([128, 1024], F32, name="p_sim", tag="sim")
                    for dh in range(2):
                        for nb in range(2):
                            nc.tensor.matmul(
                                p_sim[:, 512 * nb:512 * (nb + 1)],
                                t_on[a][:, dh, 128 * mb:128 * (mb + 1)],
                                t_on[bm][:, dh, 512 * nb:512 * (nb + 1)],
                                start=(dh == 0), stop=(dh == 1))
                    t_exp = expp.tile([128, 1024], BF16, name="t_exp", tag="exp")
                    nc.scalar.activation(t_exp[:], p_sim[:], AF.Exp, scale=10.0,
                                         accum_out=rp_st[:, bslot, mb:mb + 1])
                    if bslot > 0:
                        if mb == 0:
                            nc.vector.tensor_copy(t_cs[:], t_exp[:])
                        else:
                            nc.vector.tensor_tensor(t_cs[:], t_cs[:], t_exp[:], ALU.add)
                if bslot > 0:
                    nc.sync.dma_start(a_cs[:, 1024 * (bslot - 1):1024 * bslot], t_cs[:])

            t_h = [None] * 4
            t_sq = [None] * 4
            p_ns = [None] * 4

            def new_th(k):
                t_h[k] = wk.tile([128, 2, 1024], BF16, name=f"t_h{k}", tag="th")

            # ---- emission order tuned to keep the PE stream dense ----
            load_slab(0)
            load_slab(1)
            new_th(0)
            head_half(0, 0, t_h[0])
            head_half(0, 1, t_h[0])
            t_sq[0] = sq_of(t_h[0])
            load_slab(2)
            new_th(1)
            head_half(1, 0, t_h[1])
            p_ns[0] = ns_mm(t_sq[0])
            norm_act(0, p_ns[0], t_h[0])
            head_half(1, 1, t_h[1])
            t_sq[1] = sq_of(t_h[1])
            block(0, *BLOCKS[0])
            load_slab(3)
            new_th(2)
            head_half(2, 0, t_h[2])
            p_ns[1] = ns_mm(t_sq[1])
            norm_act(1, p_ns[1], t_h[1])
            head_half(2, 1, t_h[2])
            t_sq[2] = sq_of(t_h[2])
            block(1, *BLOCKS[1])
            new_th(3)
            head_half(3, 0, t_h[3])
            p_ns[2] = ns_mm(t_sq[2])
            norm_act(2, p_ns[2], t_h[2])
            head_half(3, 1, t_h[3])
            t_sq[3] = sq_of(t_h[3])
            p_ns[3] = ns_mm(t_sq[3])
            norm_act(3, p_ns[3], t_h[3])

            # pos: elementwise product slabs slot0 x slot3, column sums over d
            # (emitted here so it overlaps blocks 2-4 instead of tailing)
            t_pp = wk.tile([128, 2, 1024], BF16, name="t_pp", tag="pp", bufs=1)
            nc.vector.tensor_tensor(t_pp[:], t_on[0][:], t_on[3][:], ALU.mult)
            for nb in range(2):
                p_ps = headp.tile([1, 512], F32, name=f"p_ps{nb}", tag="head")
                for dh in range(2):
                    nc.tensor.matmul(p_ps[:], t_ones[:, 0:1],
                                     t_pp[:, dh, 512 * nb:512 * (nb + 1)],
                                     start=(dh == 0), stop=(dh == 1))
                nc.vector.tensor_copy(ps_st[0:1, 512 * nb:512 * (nb + 1)], p_ps[:])
            nc.sync.dma_start(a_ps[:], ps_st[:])

            block(2, *BLOCKS[2])
            block(3, *BLOCKS[3])
            block(4, *BLOCKS[4])

            # final DMA
            nc.sync.dma_start(a_rs[:], rp_st[:].rearrange("p a m -> p (a m)"))

    nc.compile()
    _CACHE["nc"] = nc
    return nc


def _host_inputs(embedded_data, W, b):
    import ml_dtypes
    bf16 = ml_dtypes.bfloat16
    embT = np.asarray(embedded_data, dtype=np.float32).T.astype(bf16)
    Wb = np.asarray(W, dtype=np.float32).astype(bf16)
    b32 = np.asarray(b, dtype=np.float32)
    ones = np.ones((128, 128), dtype=bf16)
    in_maps = []
    for c in range(8):
        cols = np.ascontiguousarray(np.concatenate(
            [embT[:, 1024 * s:1024 * (s + 1)] for s in SLOTS[c]], axis=1))
        in_maps.append({"embT": cols, "W": Wb, "b": b32, "ones": ones})
    return in_maps


def _combine(results):
    neg = np.zeros(8192, np.float64)
    pos = np.zeros(8192, np.float64)
    E10 = np.exp(10.0)
    for c in range(8):
        S = SLOTS[c]
        rs = results[c]["rowacc"].astype(np.float64).reshape(128, 5, 8)
        cs = results[c]["colacc"].astype(np.float64).reshape(128, 4, 1024).sum(axis=0)
        rows = [rs[:, bl, :].T.reshape(-1) for bl in range(5)]
        sl = [np.s_[1024 * s:1024 * (s + 1)] for s in S]
        neg[sl[0]] += rows[0] - E10          # diag block, self-sim removed
        neg[sl[0]] += rows[1]; neg[sl[1]] += cs[0]   # B1 (0,1)
        neg[sl[0]] += rows[2]; neg[sl[2]] += cs[1]   # B2 (0,2)
        neg[sl[1]] += rows[3]; neg[sl[3]] += cs[2]   # B3 (1,3)
        if c < 4:                                     # B4 (0,3) dedup: cores 0-3
            neg[sl[0]] += rows[4]; neg[sl[3]] += cs[3]
            ps = results[c]["possim"].astype(np.float64).ravel()
            pos[sl[0]] = ps
            pos[1024 * S[3]:1024 * (S[3] + 1)] = ps
    loss = -np.mean(10.0 * pos - np.log(neg))
    return np.float32(loss)


def run(embedded_data, W, b, trace=False):
    from concourse import bass_utils
    nc = _build()
    in_maps = _host_inputs(embedded_data, W, b)
    res = bass_utils.run_bass_kernel_spmd(nc, in_maps, core_ids=list(range(8)),
                                          trace=trace)
    return _combine(res.results), res


def kernel(embedded_data, W, b):
    loss, _ = run(embedded_data, W, b, trace=False)
    return np.asarray(loss, dtype=np.float32)
